# revision 1
# baseline (speedup 1.0000x reference)
"""DANet dual-attention head on 8 Trainium2 NeuronCores.

Sharding: core = 2*b + h handles batch b (of 4), row-half h (of 2).
Each core computes both PAM and CAM branches for its 32-row half
(extended by 1 halo row each side for the second 3x3 conv).
Cross-core traffic (within pairs): AllGather of (q, v) for the spatial
attention, AllReduce of the channel energy matrix for CAM.

Matmuls run in float32r (fp22 mantissa, full PE rate); PAM logits in bf16.
"""
import numpy as np
import concourse.bacc as bacc
import concourse.bass as bass
import concourse.mybir as mybir
import concourse.tile as tile
from concourse.bass_utils import run_bass_kernel_spmd
from concourse.masks import make_identity
from contextlib import ExitStack

F32 = mybir.dt.float32
F32R = mybir.dt.float32r
BF16 = mybir.dt.bfloat16
AF = mybir.ActivationFunctionType

B, CIN, CM, CK, COUT, H, W = 4, 512, 128, 16, 64, 64, 64
NCC = CIN // 128            # 4 cin chunks
EXT = 34 * 66               # padded extended layout (34 rows x 66 cols)
XF = 36 * 66 + 2            # x-halo flat + 1 guard col each side
HALF = 1122                 # EXT/2, attention i-half
REAL = 2048                 # 32 rows x 64 cols packed
PAIRS = [[0, 1], [2, 3], [4, 5], [6, 7]]
EPS = 1e-5

# weight blob layout (per-partition f32 columns)
W1_F = 72 * 128             # conv1 weights: 72 tiles of 128 (conv x offset x cinchunk)
W2_OFF = 0                  # within wmat2: 18 tiles of 128
KW_OFF = W2_OFF + 18 * 128  # [128,16]
VW_OFF = KW_OFF + 16        # [128,128]
W31_OFF = VW_OFF + 128
W32_OFF = W31_OFF + 64
W4_OFF = W32_OFF + 64
W2_F = W4_OFF + 64          # end of wmat2 region
WMAT_F = W1_F + W2_F        # end of f32r matmul region
MISC_F = 16                 # misc f32 columns
# misc cols: 0 sc1,1 sh1,2 sc2,3 sh2,4 sc21,5 sh21,6 sc22,7 sh22,
#            8 vb,9 kb,10 b31,11 b32,12 b4,13 gamma_p,14 gamma_c,15 pad

DEBUG_OUTS = False


def build_program():
    nc = bacc.Bacc("TRN2", target_bir_lowering=False, debug=False, num_devices=8)
    d_xh = nc.dram_tensor("xh", [NCC, 128, XF], F32, kind="ExternalInput").ap()
    d_wb = nc.dram_tensor("wb", [128, WMAT_F + MISC_F], F32, kind="ExternalInput").ap()
    d_mask = nc.dram_tensor("mask", [1, EXT], F32, kind="ExternalInput").ap()
    d_pa = nc.dram_tensor("pa_out", [COUT, REAL], F32, kind="ExternalOutput").ap()
    d_ca = nc.dram_tensor("ca_out", [COUT, REAL], F32, kind="ExternalOutput").ap()
    d_fs = nc.dram_tensor("fs_out", [COUT, REAL], F32, kind="ExternalOutput").ap()
    dbg = {}
    if DEBUG_OUTS:
        for nm, shp in [("feat1", [128, EXT]), ("feat2", [128, EXT]),
                        ("qful", [16, 2, REAL]), ("vful", [128, 4096]),
                        ("ensum", [128, 128]), ("paf", [128, EXT]),
                        ("caf", [128, EXT]), ("acc", [128, EXT])]:
            dbg[nm] = nc.dram_tensor("dbg_" + nm, shp, F32, kind="ExternalOutput").ap()

    # collective bounce buffers (internal DRAM)
    q_in = nc.dram_tensor("q_in", [16, REAL], BF16)
    q_out = nc.dram_tensor("q_out", [2, 16, REAL], BF16)
    v_in = nc.dram_tensor("v_in", [REAL, 128], BF16)
    v_out = nc.dram_tensor("v_out", [2, REAL, 128], BF16)
    en_in = nc.dram_tensor("en_in", [128, 128], F32)
    en_out = nc.dram_tensor("en_out", [2, 128, 128], F32)

    with tile.TileContext(nc) as tc, ExitStack() as ctx:
        big = ctx.enter_context(tc.tile_pool(name="big", bufs=2))
        w1p = ctx.enter_context(tc.tile_pool(name="w1p", bufs=1))
        pers = ctx.enter_context(tc.tile_pool(name="pers", bufs=1))
        cvo = ctx.enter_context(tc.tile_pool(name="cvo", bufs=2))
        outp = ctx.enter_context(tc.tile_pool(name="outp", bufs=2))
        ps_att = ctx.enter_context(tc.tile_pool(name="ps_att", bufs=1, space="PSUM"))
        ps_misc = ctx.enter_context(tc.tile_pool(name="ps_misc", bufs=2, space="PSUM"))
        ep = ctx.enter_context(tc.tile_pool(name="ep", bufs=8))

        # ---------- phase 0: loads ----------
        xh_a = big.tile([128, 2, XF], F32R, tag="big")
        xh_b = big.tile([128, 2, XF], F32R, tag="big")
        wmat1 = w1p.tile([128, W1_F], F32R, tag="w1")
        HW1 = W1_F // 2
        nc.sync.dma_start(out=xh_a, in_=d_xh[0:2].rearrange("c p f -> p c f").bitcast(F32R))
        nc.sync.dma_start(out=wmat1[:, :HW1], in_=d_wb[:, :HW1].bitcast(F32R))
        nc.sync.dma_start(out=xh_b, in_=d_xh[2:4].rearrange("c p f -> p c f").bitcast(F32R))
        nc.sync.dma_start(out=wmat1[:, HW1:], in_=d_wb[:, HW1:W1_F].bitcast(F32R))
        wmat2 = pers.tile([128, W2_F], F32R)
        nc.sync.dma_start(out=wmat2, in_=d_wb[:, W1_F:WMAT_F].bitcast(F32R))
        misc = pers.tile([128, MISC_F], F32)
        nc.sync.dma_start(out=misc, in_=d_wb[:, WMAT_F:])
        mask_sb = pers.tile([128, EXT], BF16)
        nc.gpsimd.dma_start(out=mask_sb, in_=d_mask[0:1, :].partition_broadcast(128))
        ident_f = pers.tile([128, 128], F32)
        make_identity(nc, ident_f)
        ident = pers.tile([128, 128], F32R)
        nc.vector.tensor_copy(ident, ident_f)
        ones_f = pers.tile([128, 1], F32)
        nc.vector.memset(ones_f, 1.0)
        ones = pers.tile([128, 1], F32R)
        nc.vector.tensor_copy(ones, ones_f)
        ones_r1f = pers.tile([1, 128], F32)
        nc.vector.memset(ones_r1f, 1.0)
        ones_r1 = pers.tile([1, 128], F32R)
        nc.vector.tensor_copy(ones_r1, ones_r1f)

        def w1T(c, o, cc):
            k = (c * 9 + o) * NCC + cc
            return wmat1[:, k * 128:(k + 1) * 128]

        def w2T(c, o):
            k = c * 9 + o
            return wmat2[:, W2_OFF + k * 128:W2_OFF + (k + 1) * 128]

        CH6 = [(k * 374, 374) for k in range(6)]

        def conv1(c, feat, sc_col, sh_col):
            # 3x3 conv of x-halo -> ext layout [128, 2244], then BN+ReLU.
            # (cc,o) outer so each weight tile is loaded once; 6 psum banks
            # accumulate all spatial chunks concurrently.
            ps = ps_att.tile([128, 6, 512], F32, tag="pso")
            for cc in range(NCC):
                src = xh_a if cc < 2 else xh_b
                for o in range(9):
                    dy, dxi = divmod(o, 3)
                    w = w1T(c, o, cc)
                    for m, (n0, nn) in enumerate(CH6):
                        base = 1 + n0 + dy * 66 + dxi - 1
                        nc.tensor.matmul(ps[:, m, :nn], w, src[:, cc % 2, base:base + nn],
                                         start=(cc == 0 and o == 0),
                                         stop=(cc == NCC - 1 and o == 8))
            for m, (n0, nn) in enumerate(CH6):
                nc.scalar.activation(feat[:, n0:n0 + nn], ps[:, m, :nn], AF.Relu,
                                     bias=misc[:, sh_col:sh_col + 1],
                                     scale=misc[:, sc_col:sc_col + 1])

        # ---------- phase 1a: conv1 (w11) -> feat1 ----------
        feat1 = pers.tile([128, EXT], F32R, tag="f1")
        conv1(0, feat1, 0, 1)
        f1v = feat1.rearrange("p (r c) -> p r c", c=66)

        # ---------- phase 1b: q/v 1x1 convs, pack, AllGather ----------
        q_ext = pers.tile([16, EXT], BF16)
        for (n0, nn) in CH6:
            ps = ps_misc.tile([16, 374], F32, tag="psm")
            nc.tensor.matmul(ps, wmat2[:, KW_OFF:KW_OFF + 16], feat1[:, n0:n0 + nn],
                             start=True, stop=True)
            nc.scalar.activation(q_ext[:, n0:n0 + nn], ps, AF.Identity,
                                 bias=misc[0:16, 9:10])
        v_ext = cvo.tile([128, EXT], BF16, tag="cvo")
        for (n0, nn) in CH6:
            ps = ps_misc.tile([128, 374], F32, tag="psm")
            nc.tensor.matmul(ps, wmat2[:, VW_OFF:VW_OFF + 128], feat1[:, n0:n0 + nn],
                             start=True, stop=True)
            nc.scalar.activation(v_ext[:, n0:n0 + nn], ps, AF.Identity,
                                 bias=misc[:, 8:9])
        qev = q_ext.rearrange("p (r c) -> p r c", c=66)
        nc.sync.dma_start(out=q_in.ap().rearrange("p (r c) -> p r c", c=64),
                          in_=qev[:, 1:33, 1:65])
        nc.gpsimd.collective_compute("AllGather", mybir.AluOpType.bypass,
                                     replica_groups=PAIRS,
                                     ins=[q_in.ap()], outs=[q_out.ap()])
        vev = v_ext.rearrange("p (r c) -> p r c", c=66)
        v_blk = cvo.tile([128, REAL], BF16, tag="cvo")
        nc.vector.transpose(v_blk.rearrange("p (r c) -> p r c", c=64),
                            vev[:, 1:33, 1:65])
        vh = v_in.handle if hasattr(v_in, 'handle') else v_in
        for pb in range(4):
            outap = bass.AP(tensor=vh, offset=32 * pb,
                            ap=[[128, 32], [4096, 64], [1, 32]])
            nc.sync.dma_start(out=outap,
                              in_=v_blk[32 * pb:32 * (pb + 1), :].rearrange(
                                  "p (fb c) -> p fb c", c=32))
        nc.gpsimd.collective_compute("AllGather", mybir.AluOpType.bypass,
                                     replica_groups=PAIRS,
                                     ins=[v_in.ap()], outs=[v_out.ap()])
        vT = pers.tile([128, 4096], BF16)
        voh = v_out.handle if hasattr(v_out, 'handle') else v_out
        nc.sync.dma_start(out=vT.rearrange("p (jc c) -> p jc c", c=128),
                          in_=bass.AP(tensor=voh, offset=0,
                                      ap=[[128, 128], [16384, 32], [1, 128]]))

        # ---------- phase 1c: conv1 (w12) -> feat2 ----------
        feat2 = pers.tile([128, EXT], F32R, tag="f2")
        conv1(1, feat2, 2, 3)
        f2v = feat2.rearrange("p (r c) -> p r c", c=66)

        # ---------- phase 1d: channel energy + AllReduce ----------
        f_packed = cvo.tile([128, REAL], F32R, tag="cvo")
        nc.vector.tensor_copy(f_packed.rearrange("p (r c) -> p r c", c=64),
                              f2v[:, 1:33, 1:65])
        fT = cvo.tile([128, REAL], F32R, tag="cvo")
        en_ps = ps_misc.tile([128, 128], F32, tag="psm")
        for t in range(16):
            tp = ps_misc.tile([128, 128], F32R, tag="psm")
            nc.tensor.transpose(tp, f_packed[:, 128 * t:128 * (t + 1)], ident)
            nc.vector.tensor_copy(fT[:, 128 * t:128 * (t + 1)], tp)
        for t in range(16):
            sl = fT[:, 128 * t:128 * (t + 1)]
            nc.tensor.matmul(en_ps, sl, sl, start=(t == 0), stop=(t == 15))
        en_sb = pers.tile([128, 128], F32)
        nc.vector.tensor_copy(en_sb, en_ps)

        # ---------- phase 2a: gather-in ----------
        q_full = pers.tile([16, 2, REAL], BF16)
        nc.sync.dma_start(out=q_full, in_=q_out.ap().rearrange("g p f -> p g f"))

        # ---------- phase 2b: PAM attention ----------
        acc = w1p.tile([128, EXT], F32R, tag="w1")
        pa_feat = pers.tile([128, EXT + 2], F32R, tag="paf")
        for ihalf in range(2):
            io = ihalf * HALF
            rd_b = outp.tile([128, HALF], F32, tag="rdb", bufs=1)
            rdg = rd_b[0:1, :].bitcast(F32R)
            ps_o = ps_att.tile([128, 3, 512], F32, tag="pso")
            for jc in range(32):
                g, lo = divmod(jc, 16)
                lhs_q = q_full[:, g, 128 * lo:128 * (lo + 1)]
                e_t = ep.tile([128, HALF], BF16, tag="e", bufs=11)
                for m in range(3):
                    psl = ps_misc.tile([128, 374], F32, tag="psm")
                    nc.tensor.matmul(psl, lhs_q, q_ext[:, io + 374 * m:io + 374 * (m + 1)],
                                     start=True, stop=True)
                    nc.scalar.activation(e_t[:, 374 * m:374 * (m + 1)], psl, AF.Exp)
                if jc == 0:
                    nc.vector.tensor_copy(acc[:, io:io + HALF], e_t)
                else:
                    nc.vector.tensor_add(acc[:, io:io + HALF], acc[:, io:io + HALF], e_t)
                for m in range(3):
                    nc.tensor.matmul(ps_o[:, m, :374], vT[:, 128 * jc:128 * (jc + 1)],
                                     e_t[:, 374 * m:374 * (m + 1)],
                                     start=(jc == 0), stop=(jc == 31))
            # normalize + residual for this half
            for m in range(3):
                dn = ps_misc.tile([1, 374], F32, tag="psm")
                nc.tensor.matmul(dn, ones, acc[:, io + 374 * m:io + 374 * (m + 1)],
                                 start=True, stop=True)
                with nc.allow_low_precision(reason="fp32r rdg feeds fp32r matmul broadcast"):
                    nc.vector.reciprocal(rdg[0:1, 374 * m:374 * (m + 1)], dn)
            nc.vector.tensor_scalar_mul(rdg, rdg, misc[0:1, 13:14])  # fold gamma_p
            for m in range(3):
                sl0 = io + 374 * m
                msl = slice(374 * m, 374 * (m + 1))
                ps_b = ps_misc.tile([128, 374], F32, tag="psm")
                nc.tensor.matmul(ps_b, ones_r1, rdg[0:1, msl], start=True, stop=True)
                nc.vector.tensor_copy(rd_b[:, msl].bitcast(F32R), ps_b)
                nc.vector.tensor_mul(pa_feat[:, 1 + sl0:1 + sl0 + 374], ps_o[:, m, :374],
                                     rd_b[:, msl])
                nc.vector.tensor_add(pa_feat[:, 1 + sl0:1 + sl0 + 374],
                                     pa_feat[:, 1 + sl0:1 + sl0 + 374],
                                     feat1[:, sl0:sl0 + 374])
        nc.vector.tensor_mul(pa_feat[:, 1:1 + EXT], pa_feat[:, 1:1 + EXT], mask_sb)

        # ---------- phase 2c: CAM (energy exchange + channel softmax) ----------
        nc.sync.dma_start(out=en_in.ap(), in_=en_sb)
        nc.gpsimd.collective_compute("AllGather", mybir.AluOpType.bypass,
                                     replica_groups=PAIRS,
                                     ins=[en_in.ap()], outs=[en_out.ap()])
        en_g = pers.tile([128, 2, 128], F32)
        nc.sync.dma_start(out=en_g, in_=en_out.ap().rearrange("g p f -> p g f"))
        en_sum = pers.tile([128, 128], F32)
        nc.vector.tensor_add(en_sum, en_g[:, 0, :], en_g[:, 1, :])
        negE = pers.tile([128, 128], F32)
        nc.vector.tensor_scalar_mul(negE, en_sum, -1.0)
        rmax = pers.tile([128, 1], F32)
        nc.vector.reduce_max(rmax, negE, axis=mybir.AxisListType.X)
        nbias = pers.tile([128, 1], F32)
        nc.vector.tensor_scalar_mul(nbias, rmax, -1.0)
        expat = pers.tile([128, 128], F32)
        nc.scalar.activation(expat, en_sum, AF.Exp, bias=nbias[:, 0:1], scale=-1.0)
        rsum = pers.tile([128, 1], F32)
        nc.vector.reduce_sum(rsum, expat, axis=mybir.AxisListType.X)
        rcp = pers.tile([128, 1], F32)
        nc.vector.reciprocal(rcp, rsum)
        nc.vector.tensor_mul(rcp, rcp, misc[:, 14:15])  # fold gamma_c
        attn_c = pers.tile([128, 128], F32R)
        nc.vector.tensor_scalar_mul(attn_c, expat, rcp[:, 0:1])
        tp = ps_misc.tile([128, 128], F32R, tag="psm")
        nc.tensor.transpose(tp, attn_c, ident)
        attn_cT = pers.tile([128, 128], F32R)
        nc.vector.tensor_copy(attn_cT, tp)
        ca_feat = pers.tile([128, EXT + 2], F32R, tag="caf")
        for (n0, nn) in CH6:
            ps = ps_misc.tile([128, 374], F32, tag="psm")
            nc.tensor.matmul(ps, attn_cT, feat2[:, n0:n0 + nn], start=True, stop=True)
            nc.vector.tensor_add(ca_feat[:, 1 + n0:1 + n0 + nn], ps, feat2[:, n0:n0 + nn])
        nc.vector.tensor_mul(ca_feat[:, 1:1 + EXT], ca_feat[:, 1:1 + EXT], mask_sb)

        # ---------- phase 3: conv2 + heads ----------
        # valid output rows are ext rows 1..33 -> flat [66, 2178), chunked 5 rows
        CH7 = [(66 + 330 * k, 330) for k in range(6)] + [(66 + 1980, 132)]
        VF = 2112  # 32 rows x 66

        def conv2(c, src_t, dst, sc_col, sh_col):
            for ci, (n0, nn) in enumerate(CH7):
                ps = ps_misc.tile([128, 374], F32, tag="psm")
                for o in range(9):
                    dy, dxi = divmod(o, 3)
                    base = 1 + n0 + (dy - 1) * 66 + (dxi - 1)
                    nc.tensor.matmul(ps[:, :nn], w2T(c, o), src_t[:, base:base + nn],
                                     start=(o == 0), stop=(o == 8))
                nc.scalar.activation(dst[:, n0 - 66:n0 - 66 + nn], ps[:, :nn], AF.Relu,
                                     bias=misc[:, sh_col:sh_col + 1],
                                     scale=misc[:, sc_col:sc_col + 1])

        pa_conv = cvo.tile([128, VF], F32R, tag="cvo")
        conv2(0, pa_feat, pa_conv, 4, 5)
        ca_conv = cvo.tile([128, VF], F32R, tag="cvo")
        conv2(1, ca_feat, ca_conv, 6, 7)
        fsum = pers.tile([128, VF], F32R, tag="f1b")
        nc.vector.tensor_add(fsum, pa_conv, ca_conv)

        def head(src_t, woff, bcol, dram):
            for ci, (n0, nn) in enumerate(CH7):
                v0 = n0 - 66
                ps = ps_misc.tile([64, 374], F32, tag="psm")
                nc.tensor.matmul(ps[:, :nn], wmat2[:, woff:woff + 64],
                                 src_t[:, v0:v0 + nn], start=True, stop=True)
                ob = outp.tile([64, 330], F32, tag="ob")
                nc.scalar.activation(ob[:, :nn], ps[:, :nn], AF.Identity,
                                     bias=misc[0:64, bcol:bcol + 1])
                nrows = nn // 66
                nc.sync.dma_start(
                    out=dram[:, (v0 // 66) * 64:(v0 // 66) * 64 + nrows * 64],
                    in_=ob[:, :nn].rearrange("p (r c) -> p r c", c=66)[:, :, 1:65])

        head(pa_conv, W31_OFF, 10, d_pa)
        head(ca_conv, W32_OFF, 11, d_ca)
        head(fsum, W4_OFF, 12, d_fs)

        if DEBUG_OUTS:
            nc.sync.dma_start(out=dbg["feat1"], in_=feat1.bitcast(F32))
            nc.sync.dma_start(out=dbg["feat2"], in_=feat2.bitcast(F32))
            nc.gpsimd.dma_start(out=dbg["qful"], in_=q_full)
            nc.gpsimd.dma_start(out=dbg["vful"], in_=vT)
            nc.sync.dma_start(out=dbg["ensum"], in_=en_sum)
            nc.sync.dma_start(out=dbg["paf"], in_=pa_feat[:, 1:1 + EXT].bitcast(F32))
            nc.sync.dma_start(out=dbg["caf"], in_=ca_feat[:, 1:1 + EXT].bitcast(F32))
            nc.sync.dma_start(out=dbg["acc"], in_=acc.bitcast(F32))

    nc.compile()
    return nc


def prep_inputs(inputs):
    """Host-side shard prep: returns in_maps for the 8 cores."""
    g = {k: np.asarray(v, dtype=np.float32) for k, v in inputs.items()}
    x = g["x"]

    def bnfold(gk, bk, mk, vk):
        sc = g[gk] / np.sqrt(g[vk] + EPS)
        sh = g[bk] - g[mk] * sc
        return sc, sh

    sc1, sh1 = bnfold("g11", "b11", "m11", "v11")
    sc2, sh2 = bnfold("g12", "b12", "m12", "v12")
    sc21, sh21 = bnfold("g21", "b21", "m21", "v21")
    sc22, sh22 = bnfold("g22", "b22", "m22", "v22")

    # weight blob
    wb = np.zeros((128, WMAT_F), np.float32)
    for c, wkey in ((0, "w11"), (1, "w12")):
        wc = g[wkey]  # [128, 512, 3, 3]
        for o in range(9):
            dy, dx = divmod(o, 3)
            for cc in range(NCC):
                k = (c * 9 + o) * NCC + cc
                wb[:, k * 128:(k + 1) * 128] = wc[:, 128 * cc:128 * (cc + 1), dy, dx].T
    for c, wkey in ((0, "w21"), (1, "w22")):
        wc = g[wkey]  # [128, 128, 3, 3]
        for o in range(9):
            dy, dx = divmod(o, 3)
            k = c * 9 + o
            wb[:, W1_F + W2_OFF + k * 128:W1_F + W2_OFF + (k + 1) * 128] = wc[:, :, dy, dx].T
    wb[:, W1_F + KW_OFF:W1_F + KW_OFF + 16] = g["pam_kw"][:, :, 0, 0].T
    wb[:, W1_F + VW_OFF:W1_F + VW_OFF + 128] = g["pam_vw"][:, :, 0, 0].T
    wb[:, W1_F + W31_OFF:W1_F + W31_OFF + 64] = g["w31"][:, :, 0, 0].T
    wb[:, W1_F + W32_OFF:W1_F + W32_OFF + 64] = g["w32"][:, :, 0, 0].T
    wb[:, W1_F + W4_OFF:W1_F + W4_OFF + 64] = g["w4"][:, :, 0, 0].T
    mc = np.zeros((128, MISC_F), np.float32)
    mc[:, 0], mc[:, 1], mc[:, 2], mc[:, 3] = sc1, sh1, sc2, sh2
    mc[:, 4], mc[:, 5], mc[:, 6], mc[:, 7] = sc21, sh21, sc22, sh22
    mc[:, 8] = g["pam_vb"]
    mc[0:16, 9] = g["pam_kb"]
    mc[0:64, 10] = g["b31"]
    mc[0:64, 11] = g["b32"]
    mc[0:64, 12] = g["b4"]
    mc[:, 13] = g["pam_gamma"][0]
    mc[:, 14] = g["cam_gamma"][0]
    wblob = np.concatenate([wb, mc], axis=1)

    in_maps = []
    for core in range(8):
        b, h = divmod(core, 2)
        xp = np.zeros((CIN, 36, 66), np.float32)
        r0 = 32 * h - 2
        lo, hi = max(0, r0), min(64, r0 + 36)
        xp[:, lo - r0:hi - r0, 1:65] = x[b, :, lo:hi, :]
        xf = xp.reshape(NCC, 128, 36 * 66)
        xf = np.pad(xf, ((0, 0), (0, 0), (1, 1)))
        mask = np.zeros((1, EXT), np.float32)
        mv = mask.reshape(1, 34, 66)
        mv[:, :, 1:65] = 1.0
        if h == 0:
            mv[:, 0, :] = 0.0   # image row -1 is fake
        else:
            mv[:, 33, :] = 0.0  # image row 64 is fake
        in_maps.append({"xh": np.ascontiguousarray(xf), "wb": wblob, "mask": mask})
    return in_maps


_NC_CACHE = {}


def _get_program():
    if "nc" not in _NC_CACHE:
        _NC_CACHE["nc"] = build_program()
    return _NC_CACHE["nc"]


def run_cores(inputs, trace=False):
    nc = _get_program()
    in_maps = prep_inputs(inputs)
    res = run_bass_kernel_spmd(nc, in_maps, list(range(8)), trace=trace)
    return res


def assemble(results):
    out = np.zeros((3, B, COUT, H, W), np.float32)
    for core in range(8):
        b, h = divmod(core, 2)
        r = results[core]
        for i, key in enumerate(("pa_out", "ca_out", "fs_out")):
            out[i, b, :, 32 * h:32 * (h + 1), :] = r[key].reshape(COUT, 32, 64)
    return out


def kernel(**inputs):
    res = run_cores(inputs, trace=False)
    return assemble(res.results)

